# revision 21
# baseline (speedup 1.0000x reference)
"""Trainium2 Bass kernel for nn_AutoregressiveGRUWithAttention.

Data-parallel over batch: 8 cores x 128 batch. Feature-on-partition layout.

v2 architecture (custom-DVE gate chain):
  r-gate:  tanh via custom DVE deg-7 Horner poly (gamma-normalized d7=-1,
           no clamp; preacts empirically |v|<0.9, fit region [0,1.75]).
  z-gate:  EXACT tanh on ACT (exp_and_others table), scale=-0.5 in-op;
           encoder length-mask rides the z preact (+BIG) -> tanh(-30) = -1
           exactly freezes h past each sequence end.
           zc2 = 0.5*K_N*(1+tz) (ACT affine) feeds the fused n-gate op.
  n-gate:  yv = CLAMP_ADD(A, tr*B') then m = TANH7M(yv, zc2) where TANH7M
           is the factored deg-7 form u*((z-a)^2+b^2)*(1-z)*Src1 with the
           real root normalized to 1 (weights pre-scaled by G_N) and the
           leading coeff K_N folded into zc2. m = 0.5*(1+tz)*n directly.
  mix:     pp = 0.5*(1-tz)*h via custom ZC_MUL on DVE; h' = m + pp.
  Attention: streamed unnormalized softmax. logits/y matmuls run on h and
  attn parts separately (o = h + attn never blocks them); e=exp on ACT
  (bf16), eo = e*o on Pool; s|AC accumulate in a persistent PSUM bank via
  two PE identity-matmuls; rec = reciprocal_approx_fast(s); attnB = AC*rec.
  PSUM column-packed banks: rz = [r | z], ab = [A | B'] (all partitions
  0:64 so every elementwise consumer is lane-legal).
"""
import numpy as np
import ml_dtypes

B, L, T, IN, H, OUT = 1024, 64, 128, 13, 64, 13
NCORES, BL = 8, 128
BIG = 60.0
BF16 = ml_dtypes.bfloat16

# ---- fitted polynomial constants (see sim.py) ----
# r,z gates: EXACT tanh on ACT. n-gate deg-7 factored on [0,1.6]:
G_N = 0.4280260088298112
A_N = 0.1851027405103682
B2_N = 0.4952995825046665
K_N = 4.3954206142704955
CLAMP_N = G_N * 1.6

# wh pack column offsets [65 x _WH_COLS] (bf16); all lhsT [K=65, 64]
_ER, _EZ, _EA, _EB = 0, 64, 128, 192
_DR, _DZ, _DA = 256, 320, 384
_AR, _AZ, _AA = 448, 512, 576
_OR, _OZ, _OA = 640, 704, 768
_WOL, _WOY, _I64 = 832, 896, 909
_WH_COLS = 973
_WX_COLS = 192  # [XR 64 | XZ 64 | XA 64], K = 14 (13 features + invm row)

LAST_EXEC_NS = None
TRACE = False
TRACE_DIR = None
WARM_DUMMIES = 0

_OPS = {}


def _register_ops():
    """Define + register the custom DVE ops (idempotent per process)."""
    global _OPS
    if _OPS:
        return _OPS
    import concourse.dve_ops as dve_ops
    from concourse.dve_spec import (Spec, Src0, Src1, C0, C1, C2, Zero, One,
                                    maxx, minn, sq, lower, _has_src1)
    from concourse.dve_uop import DveOpSpec

    # out = (x*((z-C0)^2 + C1)) * (1-z), z = x^2: factored deg-7 tanh/K_N
    # (real root normalized to 1 via G_N weight prescale); K_N rides the
    # h'-mix stt scalar.
    _zm = sq(Src0)
    TANH7P = Spec(
        body=(Src0 * (sq(_zm - C0) + C1)) * (One - _zm),
        reference=lambda in0, in1, s0, s1, imm2: (
            lambda u, z: u * ((z - s0) ** 2 + s1) * (1.0 - z))(
                in0.astype(np.float32), in0.astype(np.float32) ** 2),
    )

    CLAMP_ADD = Spec(
        body=minn(maxx(Src0 + Src1, Zero - C2), C2),
        reference=lambda in0, in1, s0, s1, imm2: np.clip(
            in0.astype(np.float32) + in1, -imm2, imm2),
    )

    # out = (C0 - Src0) * Src1 * C1     (pp = 0.5*(1-tz)*h)
    ZC_MUL = Spec(
        body=(C0 - Src0) * Src1 * C1,
        reference=lambda in0, in1, s0, s1, imm2: (
            (s0 - in0.astype(np.float32)) * in1 * s1),
    )

    specs = {"ANT_TANH7P": TANH7P,
             "ANT_CLAMP_ADD": CLAMP_ADD, "ANT_ZC_MUL": ZC_MUL}
    ops = {}
    for name, spec in specs.items():
        if name in dve_ops._SUB_OPCODE_FOR_NAME:
            ops[name] = next(o for o in dve_ops.OPS if o.name == name)
            continue
        row = max(dve_ops._SUB_OPCODE_FOR_NAME.values()) + 1
        assert row < 0x20, "custom-DVE row overflow"
        dve_ops._SUB_OPCODE_FOR_NAME[name] = row
        shas = {}
        for ver in ("v3", "v4"):
            try:
                uops = lower(spec, ver=ver)
            except Exception:
                continue
            shas[ver] = DveOpSpec(name=name, opcode=row, uops=uops,
                                  rd1_en=_has_src1(spec)).sha(ver)
        op = dve_ops.DveOp(name, spec, False, shas,
                           perf_en={"v3": True, "v4": True}
                           if name == "ANT_TANH7P" else {})
        dve_ops.OPS.append(op)
        dve_ops.CUSTOM_DVE_SPECS[name] = spec
        ops[name] = op
    _OPS = ops
    return ops


def _prep_weights(Wih, Whh, bih, bhh, Wf, bf, Wa, ba):
    f8 = np.float64
    Wih, Whh, bih, bhh, Wf, bf, Wa, ba = [np.asarray(a, f8) for a in
                                          (Wih, Whh, bih, bhh, Wf, bf, Wa, ba)]
    Wr, Wz, Wn = Wih[0:H], Wih[H:2 * H], Wih[2 * H:3 * H]
    Ur, Uz, Un = Whh[0:H], Whh[H:2 * H], Whh[2 * H:3 * H]
    br_i, bz_i, bn_i = bih[0:H], bih[H:2 * H], bih[2 * H:3 * H]
    br_h, bz_h, bn_h = bhh[0:H], bhh[H:2 * H], bhh[2 * H:3 * H]
    CR, CZ, CA = Wf.T @ Wr.T, Wf.T @ Wz.T, Wf.T @ Wn.T
    cr_b, cz_b, ca_b = bf @ Wr.T, bf @ Wz.T, bf @ Wn.T

    def blk(w, bias, scale):
        m = np.zeros((H + 1, H), f8)
        m[0:H] = scale * w
        m[H] = scale * bias
        return m

    wh = np.zeros((H + 1, _WH_COLS), f8)
    wh[:, _ER:_ER + H] = blk(Ur.T, br_i + br_h, 1.0)
    wh[:, _EZ:_EZ + H] = blk(Uz.T, bz_i + bz_h, 1.0)
    wh[:, _EA:_EA + H] = blk(0.5 * Un.T, bn_i + 0.5 * bn_h, G_N)
    wh[:, _EB:_EB + H] = blk(0.5 * Un.T, 0.5 * bn_h, G_N)
    wh[:, _DR:_DR + H] = blk(Ur.T + CR, br_i + br_h + cr_b, 1.0)
    wh[:, _DZ:_DZ + H] = blk(Uz.T + CZ, bz_i + bz_h + cz_b, 1.0)
    wh[:, _DA:_DA + H] = blk(0.5 * Un.T + CA, bn_i + ca_b + 0.5 * bn_h, G_N)
    wh[:, _AR:_AR + H] = blk(CR, 0 * cr_b, 1.0)
    wh[:, _AZ:_AZ + H] = blk(CZ, 0 * cz_b, 1.0)
    wh[:, _AA:_AA + H] = blk(CA, 0 * ca_b, G_N)
    wh[:, _OR:_OR + H] = blk(CR, cr_b, 1.0)
    wh[:, _OZ:_OZ + H] = blk(CZ, cz_b, 1.0)
    wh[:, _OA:_OA + H] = blk(CA, ca_b, G_N)
    wh[:, _WOL:_WOL + H] = blk(Wa.T, ba, 1.0)
    wh[0:H, _WOY:_WOY + OUT] = Wf.T
    wh[H, _WOY:_WOY + OUT] = bf
    wh[0:H, _I64:_I64 + H] = np.eye(H)

    wx = np.zeros((IN + 1, _WX_COLS), f8)
    wx[0:IN, 0:H] = Wr.T
    wx[0:IN, H:2 * H] = Wz.T
    wx[IN, H:2 * H] = BIG                      # invm mask row (z only)
    wx[0:IN, 2 * H:3 * H] = G_N * Wn.T

    return dict(wh=np.ascontiguousarray(wh, BF16),
                wx=np.ascontiguousarray(wx, BF16))


def _prep_core(x_core, len_core):
    x_core = np.asarray(x_core, np.float32)
    xT = np.zeros((IN + 1, L, BL), np.float32)
    xT[0:IN] = np.transpose(x_core, (2, 1, 0))
    valid = (np.arange(L)[:, None] < np.asarray(len_core)[None, :])
    xT[IN] = 1.0 - valid.astype(np.float32)
    m63 = valid[L - 1].astype(np.float32)
    m63bc = np.ascontiguousarray(np.broadcast_to(m63, (H, BL)), BF16)
    return (np.ascontiguousarray(xT.reshape(IN + 1, L * BL), BF16), m63bc)


def build_nc(l_steps=L, t_steps=T, compile=True):
    ops = _register_ops()
    TANH7P = ops["ANT_TANH7P"]
    CLAMP_ADD, ZC_MUL = ops["ANT_CLAMP_ADD"], ops["ANT_ZC_MUL"]
    import concourse.bacc as bacc
    import concourse.tile as tile
    from concourse import mybir
    from contextlib import ExitStack

    f32 = mybir.dt.float32
    bf = mybir.dt.bfloat16
    AF = mybir.ActivationFunctionType
    OP = mybir.AluOpType

    nc = bacc.Bacc("TRN2", target_bir_lowering=False, debug=False,
                   num_devices=NCORES)
    d_xT = nc.declare_dram_parameter("xT", [IN + 1, l_steps * BL], bf, isOutput=False)
    d_m63 = nc.declare_dram_parameter("m63", [H, BL], bf, isOutput=False)
    d_wh = nc.declare_dram_parameter("wh", [H + 1, _WH_COLS], bf, isOutput=False)
    d_wx = nc.declare_dram_parameter("wx", [IN + 1, _WX_COLS], bf, isOutput=False)
    d_out = nc.declare_dram_parameter("out", [BL, t_steps * OUT], f32, isOutput=True)

    with tile.TileContext(nc) as tc, ExitStack() as ctx:
        const = ctx.enter_context(tc.tile_pool(name="const", bufs=1))
        temps = ctx.enter_context(tc.tile_pool(name="temps", bufs=3))
        pgate = ctx.enter_context(tc.tile_pool(name="pgate", bufs=1))
        p_r = ctx.enter_context(tc.tile_pool(name="p_r", bufs=1, space="PSUM"))
        p_z = ctx.enter_context(tc.tile_pool(name="p_z", bufs=1, space="PSUM"))
        p_a = ctx.enter_context(tc.tile_pool(name="p_a", bufs=1, space="PSUM"))
        p_b = ctx.enter_context(tc.tile_pool(name="p_b", bufs=1, space="PSUM"))
        p_l = ctx.enter_context(tc.tile_pool(name="p_l", bufs=2, space="PSUM"))
        p_y = ctx.enter_context(tc.tile_pool(name="p_y", bufs=1, space="PSUM"))
        p_sacc = ctx.enter_context(tc.tile_pool(name="p_sacc", bufs=1, space="PSUM"))

        xT = const.tile([IN + 1, l_steps * BL], bf)
        m63 = const.tile([H, BL], bf)
        wh = const.tile([H + 1, _WH_COLS], bf)
        wx = const.tile([IN + 1, _WX_COLS], bf)
        hB = const.tile([H + 1, BL], bf)
        oB = const.tile([H + 1, BL], bf)
        attnB = const.tile([H + 1, BL], bf)
        eeo = const.tile([H, 2 * BL], bf)          # [e | eo]
        out_sb = const.tile([BL, t_steps * OUT], f32)
        b05 = const.tile([H, 1], f32)
        sacc = p_sacc.tile([H, 2 * BL], f32)       # [s | AC] persistent

        for dst, srcd in ((xT, d_xT), (m63, d_m63), (wh, d_wh), (wx, d_wx)):
            nc.sync.dma_start(out=dst, in_=srcd[:])

        nc.vector.memset(hB[:], 0.0)
        nc.vector.memset(hB[H:H + 1, :], 1.0)
        nc.vector.memset(oB[H:H + 1, :], 1.0)
        nc.vector.memset(attnB[:], 0.0)
        nc.vector.memset(b05, 0.5)
        h64 = hB[0:H, :]

        def mm(out_ap, col, rhs, start, stop):
            nc.tensor.matmul(out_ap, wh[:, col:col + H], rhs[:],
                             start=start, stop=stop)

        def act_pp(tz):
            """pp = 0.5*(1-tz)*h via ACT affine + Pool mult (off the DVE)."""
            up = temps.tile([H, BL], bf, tag="up")
            nc.scalar.activation(out=up, in_=tz, func=AF.Identity,
                                 bias=b05[:], scale=-0.5)
            pp = temps.tile([H, BL], bf, tag="pp")
            nc.gpsimd.tensor_mul(out=pp, in0=up, in1=h64)
            return pp

        def dve_chain(rp, zp, ap_, bp, tr, tz, pp):
            """t2 -> yv -> n' -> m~ -> h' (DVE, partitions 0:64)."""
            t2 = temps.tile([H, BL], f32, tag="t2")
            nc.vector.tensor_mul(out=t2, in0=tr, in1=bp[:])
            yv = temps.tile([H, BL], bf, tag="yv")
            nc.vector._custom_dve(CLAMP_ADD, out=yv, in0=ap_[:], in1=t2,
                                  imm2=CLAMP_N)
            nn = temps.tile([H, BL], bf, tag="nn")
            nc.vector._custom_dve(TANH7P, out=nn, in0=yv,
                                  s0=A_N, s1=B2_N)
            mt = pgate.tile([H, BL], f32, tag="g")
            nc.vector.scalar_tensor_tensor(out=mt, in0=tz, scalar=1.0,
                                           in1=nn, op0=OP.add, op1=OP.mult)
            nc.vector.scalar_tensor_tensor(out=h64, in0=mt, scalar=0.5 * K_N,
                                           in1=pp, op0=OP.mult, op1=OP.add)

        # ================= encoder =================
        for t in range(l_steps):
            rp = p_r.tile([H, BL], f32, tag="r")
            zp = p_z.tile([H, BL], f32, tag="z")
            ap_ = p_a.tile([H, BL], f32, tag="a")
            bp = p_b.tile([H, BL], f32, tag="b")
            sl = slice(t * BL, (t + 1) * BL)
            nc.tensor.matmul(rp[:], wx[:, 0:H], xT[:, sl], start=True, stop=False)
            mm(rp[:], _ER, hB, False, True)
            tr = temps.tile([H, BL], bf, tag="tr")
            nc.scalar.activation(out=tr, in_=rp[:], func=AF.Tanh, scale=0.5)
            nc.tensor.matmul(zp[:], wx[:, H:2 * H], xT[:, sl], start=True, stop=False)
            mm(zp[:], _EZ, hB, False, True)
            tz = temps.tile([H, BL], bf, tag="tz")
            nc.scalar.activation(out=tz, in_=zp[:], func=AF.Tanh, scale=-0.5)
            nc.tensor.matmul(ap_[:], wx[:, 2 * H:3 * H], xT[:, sl], start=True, stop=False)
            mm(ap_[:], _EA, hB, False, True)
            mm(bp[:], _EB, hB, True, True)
            pp = act_pp(tz)
            dve_chain(rp, zp, ap_, bp, tr, tz, pp)
            if t == l_steps - 1:
                nc.gpsimd.tensor_mul(out=oB[0:H, :], in0=m63, in1=h64)

        # ================= decoder =================
        for t in range(t_steps):
            rp = p_r.tile([H, BL], f32, tag="r")
            zp = p_z.tile([H, BL], f32, tag="z")
            ap_ = p_a.tile([H, BL], f32, tag="a")
            bp = p_b.tile([H, BL], f32, tag="b")
            if t == 0:
                mm(rp[:], _OR, oB, True, False)
                mm(rp[:], _ER, hB, False, True)
                tr = temps.tile([H, BL], bf, tag="tr")
                nc.scalar.activation(out=tr, in_=rp[:], func=AF.Tanh, scale=0.5)
                mm(zp[:], _OZ, oB, True, False)
                mm(zp[:], _EZ, hB, False, True)
                tz = temps.tile([H, BL], bf, tag="tz")
                nc.scalar.activation(out=tz, in_=zp[:], func=AF.Tanh, scale=-0.5)
                mm(ap_[:], _OA, oB, True, False)
                mm(ap_[:], _EA, hB, False, True)
                mm(bp[:], _EB, hB, True, True)
                pp = act_pp(tz)
                dve_chain(rp, zp, ap_, bp, tr, tz, pp)
                nc.vector.tensor_copy(out=oB[0:H, :], in_=h64)
                continue
            lt = p_l.tile([H, BL], f32, tag="lt")
            yt = p_y.tile([BL, OUT], f32, tag="yt")
            # r group + tanh_r
            mm(rp[:], _DR, hB, True, False)
            mm(rp[:], _AR, attnB, False, True)
            tr = temps.tile([H, BL], bf, tag="tr")
            nc.scalar.activation(out=tr, in_=rp[:], func=AF.Tanh, scale=0.5)
            # logits for o(t-1) + exp
            nc.tensor.matmul(lt[:], wh[:, _WOL:_WOL + H], hB[:],
                             start=True, stop=False)
            nc.tensor.matmul(lt[:], wh[:, _WOL:_WOL + H], attnB[:],
                             start=False, stop=True)
            nc.scalar.activation(out=eeo[:, 0:BL], in_=lt[:], func=AF.Exp)
            # z group + tanh_z
            mm(zp[:], _DZ, hB, True, False)
            mm(zp[:], _AZ, attnB, False, True)
            tz = temps.tile([H, BL], bf, tag="tz")
            nc.scalar.activation(out=tz, in_=zp[:], func=AF.Tanh, scale=-0.5)
            pp = act_pp(tz)
            # B then A groups
            mm(bp[:], _EB, hB, True, True)
            mm(ap_[:], _DA, hB, True, False)
            mm(ap_[:], _AA, attnB, False, True)
            # eo on Pool; s-accumulate early (rec path)
            nc.gpsimd.tensor_mul(out=eeo[:, BL:2 * BL], in0=eeo[:, 0:BL],
                                 in1=oB[0:H, :])
            if t == 1:
                nc.tensor.matmul(sacc[:], wh[0:H, _I64:_I64 + H],
                                 eeo[:], start=True, stop=True,
                                 skip_group_check=True)
            else:
                nc.tensor.matmul(sacc[:, 0:BL], wh[0:H, _I64:_I64 + H],
                                 eeo[:, 0:BL], start=False, stop=True,
                                 skip_group_check=True)
            # y head for o(t-1)
            nc.tensor.matmul(yt[:], hB[:], wh[:, _WOY:_WOY + OUT],
                             start=True, stop=False)
            nc.tensor.matmul(yt[:], attnB[:], wh[:, _WOY:_WOY + OUT],
                             start=False, stop=True)
            if t > 1:
                nc.tensor.matmul(sacc[:, BL:2 * BL], wh[0:H, _I64:_I64 + H],
                                 eeo[:, BL:2 * BL], start=False, stop=True,
                                 skip_group_check=True)
            # DVE chain then attention tail
            dve_chain(rp, zp, ap_, bp, tr, tz, pp)
            rec = pgate.tile([H, BL], f32, tag="g")
            nc.vector.reciprocal_approx_fast(out=rec, in_=sacc[:, 0:BL])
            nc.vector.tensor_mul(out=attnB[0:H, :], in0=sacc[:, BL:2 * BL],
                                 in1=rec)
            nc.gpsimd.tensor_add(out=oB[0:H, :], in0=h64, in1=attnB[0:H, :])
            nc.scalar.copy(out=out_sb[:, (t - 1) * OUT:t * OUT], in_=yt[:])
        # final y for t = T-1
        yt = p_y.tile([BL, OUT], f32, tag="yt")
        nc.tensor.matmul(yt[:], hB[:], wh[:, _WOY:_WOY + OUT],
                         start=True, stop=False)
        nc.tensor.matmul(yt[:], attnB[:], wh[:, _WOY:_WOY + OUT],
                         start=False, stop=True)
        nc.scalar.copy(out=out_sb[:, (t_steps - 1) * OUT:t_steps * OUT],
                       in_=yt[:])

        nc.sync.dma_start(out=d_out[:], in_=out_sb)
    if compile:
        nc.compile()
    return nc


def _make_in_maps(inputs):
    x = np.asarray(inputs["x"], np.float32)
    lengths = np.asarray(inputs["lengths"])
    w = _prep_weights(inputs["Wih"], inputs["Whh"], inputs["bih"],
                      inputs["bhh"], inputs["Wf"], inputs["bf"],
                      inputs["Wa"], inputs["ba"])
    in_maps = []
    for c in range(NCORES):
        sl = slice(c * BL, (c + 1) * BL)
        xT, m63 = _prep_core(x[sl], lengths[sl])
        in_maps.append(dict(xT=xT, m63=m63, **w))
    return in_maps


def kernel(**inputs):
    global LAST_EXEC_NS, TRACE_DIR
    from concourse.bass_utils import run_bass_kernel_spmd
    t_steps = int(inputs.get("output_length", T))
    assert t_steps == T, f"hardcoded for output_length={T}, got {t_steps}"
    nc = build_nc()
    in_maps = _make_in_maps(inputs)
    kw = {}
    if TRACE:
        import tempfile
        TRACE_DIR = tempfile.mkdtemp(prefix="bass_trace_")
        kw = dict(trace=True, tmpdir=TRACE_DIR)
    res = None
    for attempt in range(3):
        try:
            res = run_bass_kernel_spmd(nc, in_maps, list(range(NCORES)), **kw)
            break
        except Exception:
            if attempt == 2:
                raise
    LAST_EXEC_NS = res.exec_time_ns
    outs = [np.asarray(res.results[c]["out"]).reshape(BL, T, OUT)
            for c in range(NCORES)]
    return np.concatenate(outs, axis=0)


# revision 22
# speedup vs baseline: 1.0412x; 1.0412x over previous
"""Trainium2 Bass kernel for nn_AutoregressiveGRUWithAttention.

Data-parallel over batch: 8 cores x 128 batch. Feature-on-partition layout.

v2 architecture (custom-DVE gate chain):
  r-gate:  tanh via custom DVE deg-7 Horner poly (gamma-normalized d7=-1,
           no clamp; preacts empirically |v|<0.9, fit region [0,1.75]).
  z-gate:  EXACT tanh on ACT (exp_and_others table), scale=-0.5 in-op;
           encoder length-mask rides the z preact (+BIG) -> tanh(-30) = -1
           exactly freezes h past each sequence end.
           zc2 = 0.5*K_N*(1+tz) (ACT affine) feeds the fused n-gate op.
  n-gate:  yv = CLAMP_ADD(A, tr*B') then m = TANH7M(yv, zc2) where TANH7M
           is the factored deg-7 form u*((z-a)^2+b^2)*(1-z)*Src1 with the
           real root normalized to 1 (weights pre-scaled by G_N) and the
           leading coeff K_N folded into zc2. m = 0.5*(1+tz)*n directly.
  mix:     pp = 0.5*(1-tz)*h via custom ZC_MUL on DVE; h' = m + pp.
  Attention: streamed unnormalized softmax. logits/y matmuls run on h and
  attn parts separately (o = h + attn never blocks them); e=exp on ACT
  (bf16), eo = e*o on Pool; s|AC accumulate in a persistent PSUM bank via
  two PE identity-matmuls; rec = reciprocal_approx_fast(s); attnB = AC*rec.
  PSUM column-packed banks: rz = [r | z], ab = [A | B'] (all partitions
  0:64 so every elementwise consumer is lane-legal).
"""
import numpy as np
import ml_dtypes

B, L, T, IN, H, OUT = 1024, 64, 128, 13, 64, 13
NCORES, BL = 8, 128
BIG = 60.0
BF16 = ml_dtypes.bfloat16

# ---- fitted polynomial constants (see sim.py) ----
# r,z gates: EXACT tanh on ACT. n-gate deg-7 factored on [0,1.6]:
G_N = 0.4280260088298112
A_N = 0.1851027405103682
B2_N = 0.4952995825046665
K_N = 4.3954206142704955
CLAMP_N = G_N * 1.6

# wh pack column offsets [65 x _WH_COLS] (bf16); all lhsT [K=65, 64]
_ER, _EZ, _EA, _EB = 0, 64, 128, 192
_DR, _DZ, _DA = 256, 320, 384
_AR, _AZ, _AA = 448, 512, 576
_OR, _OZ, _OA = 640, 704, 768
_WOL, _WOY, _I64 = 832, 896, 909
_WH_COLS = 973
_WX_COLS = 192  # [XR 64 | XZ 64 | XA 64], K = 14 (13 features + invm row)

LAST_EXEC_NS = None
TRACE = False
TRACE_DIR = None
WARM_DUMMIES = 0

_OPS = {}


def _register_ops():
    """Define + register the custom DVE ops (idempotent per process)."""
    global _OPS
    if _OPS:
        return _OPS
    import concourse.dve_ops as dve_ops
    from concourse.dve_spec import (Spec, Src0, Src1, C0, C1, C2, Zero, One,
                                    maxx, minn, sq, lower, _has_src1)
    from concourse.dve_uop import DveOpSpec

    # out = (x*((z-C0)^2 + C1)) * (1-z), z = x^2: factored deg-7 tanh/K_N
    # (real root normalized to 1 via G_N weight prescale); K_N rides the
    # h'-mix stt scalar.
    _zm = sq(Src0)
    TANH7P = Spec(
        body=(Src0 * (sq(_zm - C0) + C1)) * (One - _zm),
        reference=lambda in0, in1, s0, s1, imm2: (
            lambda u, z: u * ((z - s0) ** 2 + s1) * (1.0 - z))(
                in0.astype(np.float32), in0.astype(np.float32) ** 2),
    )

    CLAMP_ADD = Spec(
        body=minn(maxx(Src0 + Src1, Zero - C2), C2),
        reference=lambda in0, in1, s0, s1, imm2: np.clip(
            in0.astype(np.float32) + in1, -imm2, imm2),
    )

    # out = (C0 - Src0) * Src1 * C1     (pp = 0.5*(1-tz)*h)
    ZC_MUL = Spec(
        body=(C0 - Src0) * Src1 * C1,
        reference=lambda in0, in1, s0, s1, imm2: (
            (s0 - in0.astype(np.float32)) * in1 * s1),
    )

    specs = {"ANT_TANH7P": TANH7P,
             "ANT_CLAMP_ADD": CLAMP_ADD, "ANT_ZC_MUL": ZC_MUL}
    ops = {}
    for name, spec in specs.items():
        if name in dve_ops._SUB_OPCODE_FOR_NAME:
            ops[name] = next(o for o in dve_ops.OPS if o.name == name)
            continue
        row = max(dve_ops._SUB_OPCODE_FOR_NAME.values()) + 1
        assert row < 0x20, "custom-DVE row overflow"
        dve_ops._SUB_OPCODE_FOR_NAME[name] = row
        shas = {}
        for ver in ("v3", "v4"):
            try:
                uops = lower(spec, ver=ver)
            except Exception:
                continue
            shas[ver] = DveOpSpec(name=name, opcode=row, uops=uops,
                                  rd1_en=_has_src1(spec)).sha(ver)
        op = dve_ops.DveOp(name, spec, False, shas,
                           perf_en={"v3": True, "v4": True}
                           if name == "ANT_TANH7P" else {})
        dve_ops.OPS.append(op)
        dve_ops.CUSTOM_DVE_SPECS[name] = spec
        ops[name] = op
    _OPS = ops
    return ops


def _prep_weights(Wih, Whh, bih, bhh, Wf, bf, Wa, ba):
    f8 = np.float64
    Wih, Whh, bih, bhh, Wf, bf, Wa, ba = [np.asarray(a, f8) for a in
                                          (Wih, Whh, bih, bhh, Wf, bf, Wa, ba)]
    Wr, Wz, Wn = Wih[0:H], Wih[H:2 * H], Wih[2 * H:3 * H]
    Ur, Uz, Un = Whh[0:H], Whh[H:2 * H], Whh[2 * H:3 * H]
    br_i, bz_i, bn_i = bih[0:H], bih[H:2 * H], bih[2 * H:3 * H]
    br_h, bz_h, bn_h = bhh[0:H], bhh[H:2 * H], bhh[2 * H:3 * H]
    CR, CZ, CA = Wf.T @ Wr.T, Wf.T @ Wz.T, Wf.T @ Wn.T
    cr_b, cz_b, ca_b = bf @ Wr.T, bf @ Wz.T, bf @ Wn.T

    def blk(w, bias, scale):
        m = np.zeros((H + 1, H), f8)
        m[0:H] = scale * w
        m[H] = scale * bias
        return m

    wh = np.zeros((H + 1, _WH_COLS), f8)
    wh[:, _ER:_ER + H] = blk(Ur.T, br_i + br_h, 1.0)
    wh[:, _EZ:_EZ + H] = blk(Uz.T, bz_i + bz_h, 1.0)
    wh[:, _EA:_EA + H] = blk(0.5 * Un.T, bn_i + 0.5 * bn_h, G_N)
    wh[:, _EB:_EB + H] = blk(0.5 * Un.T, 0.5 * bn_h, G_N)
    wh[:, _DR:_DR + H] = blk(Ur.T + CR, br_i + br_h + cr_b, 1.0)
    wh[:, _DZ:_DZ + H] = blk(Uz.T + CZ, bz_i + bz_h + cz_b, 1.0)
    wh[:, _DA:_DA + H] = blk(0.5 * Un.T + CA, bn_i + ca_b + 0.5 * bn_h, G_N)
    wh[:, _AR:_AR + H] = blk(CR, 0 * cr_b, 1.0)
    wh[:, _AZ:_AZ + H] = blk(CZ, 0 * cz_b, 1.0)
    wh[:, _AA:_AA + H] = blk(CA, 0 * ca_b, G_N)
    wh[:, _OR:_OR + H] = blk(CR, cr_b, 1.0)
    wh[:, _OZ:_OZ + H] = blk(CZ, cz_b, 1.0)
    wh[:, _OA:_OA + H] = blk(CA, ca_b, G_N)
    wh[:, _WOL:_WOL + H] = blk(Wa.T, ba, 1.0)
    wh[0:H, _WOY:_WOY + OUT] = Wf.T
    wh[H, _WOY:_WOY + OUT] = bf
    wh[0:H, _I64:_I64 + H] = np.eye(H)

    wx = np.zeros((IN + 1, _WX_COLS), f8)
    wx[0:IN, 0:H] = Wr.T
    wx[0:IN, H:2 * H] = Wz.T
    wx[IN, H:2 * H] = BIG                      # invm mask row (z only)
    wx[0:IN, 2 * H:3 * H] = G_N * Wn.T

    return dict(wh=np.ascontiguousarray(wh, BF16),
                wx=np.ascontiguousarray(wx, BF16))


def _prep_core(x_core, len_core):
    x_core = np.asarray(x_core, np.float32)
    xT = np.zeros((IN + 1, L, BL), np.float32)
    xT[0:IN] = np.transpose(x_core, (2, 1, 0))
    valid = (np.arange(L)[:, None] < np.asarray(len_core)[None, :])
    xT[IN] = 1.0 - valid.astype(np.float32)
    m63 = valid[L - 1].astype(np.float32)
    m63bc = np.ascontiguousarray(np.broadcast_to(m63, (H, BL)), BF16)
    return (np.ascontiguousarray(xT.reshape(IN + 1, L * BL), BF16), m63bc)


def build_nc(l_steps=L, t_steps=T, compile=True):
    ops = _register_ops()
    TANH7P = ops["ANT_TANH7P"]
    CLAMP_ADD, ZC_MUL = ops["ANT_CLAMP_ADD"], ops["ANT_ZC_MUL"]
    import concourse.bacc as bacc
    import concourse.tile as tile
    from concourse import mybir
    from contextlib import ExitStack

    f32 = mybir.dt.float32
    bf = mybir.dt.bfloat16
    AF = mybir.ActivationFunctionType
    OP = mybir.AluOpType

    nc = bacc.Bacc("TRN2", target_bir_lowering=False, debug=False,
                   num_devices=NCORES)
    d_xT = nc.declare_dram_parameter("xT", [IN + 1, l_steps * BL], bf, isOutput=False)
    d_m63 = nc.declare_dram_parameter("m63", [H, BL], bf, isOutput=False)
    d_wh = nc.declare_dram_parameter("wh", [H + 1, _WH_COLS], bf, isOutput=False)
    d_wx = nc.declare_dram_parameter("wx", [IN + 1, _WX_COLS], bf, isOutput=False)
    d_out = nc.declare_dram_parameter("out", [BL, t_steps * OUT], f32, isOutput=True)

    with tile.TileContext(nc) as tc, ExitStack() as ctx:
        const = ctx.enter_context(tc.tile_pool(name="const", bufs=1))
        temps = ctx.enter_context(tc.tile_pool(name="temps", bufs=3))
        p_r = ctx.enter_context(tc.tile_pool(name="p_r", bufs=1, space="PSUM"))
        p_z = ctx.enter_context(tc.tile_pool(name="p_z", bufs=1, space="PSUM"))
        p_a = ctx.enter_context(tc.tile_pool(name="p_a", bufs=1, space="PSUM"))
        p_b = ctx.enter_context(tc.tile_pool(name="p_b", bufs=1, space="PSUM"))
        p_l = ctx.enter_context(tc.tile_pool(name="p_l", bufs=2, space="PSUM"))
        p_y = ctx.enter_context(tc.tile_pool(name="p_y", bufs=1, space="PSUM"))
        p_sacc = ctx.enter_context(tc.tile_pool(name="p_sacc", bufs=1, space="PSUM"))

        xT = const.tile([IN + 1, l_steps * BL], bf)
        m63 = const.tile([H, BL], bf)
        wh = const.tile([H + 1, _WH_COLS], bf)
        wx = const.tile([IN + 1, _WX_COLS], bf)
        hB = const.tile([H + 1, BL], bf)
        oB = const.tile([H + 1, BL], bf)
        attnB = const.tile([H + 1, BL], bf)
        eeo = const.tile([H, 2 * BL], bf)          # [e | eo]
        out_sb = const.tile([BL, t_steps * OUT], f32)
        b05 = const.tile([H, 1], f32)
        sacc = p_sacc.tile([H, 2 * BL], f32)       # [s | AC] persistent

        for dst, srcd in ((xT, d_xT), (m63, d_m63), (wh, d_wh), (wx, d_wx)):
            nc.sync.dma_start(out=dst, in_=srcd[:])

        nc.vector.memset(hB[:], 0.0)
        nc.vector.memset(hB[H:H + 1, :], 1.0)
        nc.vector.memset(oB[H:H + 1, :], 1.0)
        nc.vector.memset(attnB[:], 0.0)
        nc.vector.memset(b05, 0.5)
        h64 = hB[0:H, :]

        def mm(out_ap, col, rhs, start, stop):
            nc.tensor.matmul(out_ap, wh[:, col:col + H], rhs[:],
                             start=start, stop=stop)

        def act_pp(tz):
            """pp = 0.5*(1-tz)*h via ACT affine + Pool mult (off the DVE)."""
            up = temps.tile([H, BL], bf, tag="up")
            nc.scalar.activation(out=up, in_=tz, func=AF.Identity,
                                 bias=b05[:], scale=-0.5)
            pp = temps.tile([H, BL], bf, tag="pp")
            nc.gpsimd.tensor_mul(out=pp, in0=up, in1=h64)
            return pp

        def dve_chain(rp, zp, ap_, bp, tr, tz, pp):
            """t2 -> yv -> n' -> m~ -> h' (DVE, partitions 0:64)."""
            t2 = temps.tile([H, BL], f32, tag="t2")
            nc.vector.tensor_mul(out=t2, in0=tr, in1=bp[:])
            yv = temps.tile([H, BL], bf, tag="yv")
            nc.vector._custom_dve(CLAMP_ADD, out=yv, in0=ap_[:], in1=t2,
                                  imm2=CLAMP_N)
            nn = temps.tile([H, BL], bf, tag="nn")
            nc.vector._custom_dve(TANH7P, out=nn, in0=yv,
                                  s0=A_N, s1=B2_N)
            mt = temps.tile([H, BL], bf, tag="mt")
            nc.vector.scalar_tensor_tensor(out=mt, in0=tz, scalar=1.0,
                                           in1=nn, op0=OP.add, op1=OP.mult)
            nc.vector.scalar_tensor_tensor(out=h64, in0=mt, scalar=0.5 * K_N,
                                           in1=pp, op0=OP.mult, op1=OP.add)

        # ================= encoder =================
        for t in range(l_steps):
            rp = p_r.tile([H, BL], f32, tag="r")
            zp = p_z.tile([H, BL], f32, tag="z")
            ap_ = p_a.tile([H, BL], f32, tag="a")
            bp = p_b.tile([H, BL], f32, tag="b")
            sl = slice(t * BL, (t + 1) * BL)
            nc.tensor.matmul(rp[:], wx[:, 0:H], xT[:, sl], start=True, stop=False)
            mm(rp[:], _ER, hB, False, True)
            tr = temps.tile([H, BL], bf, tag="tr")
            nc.scalar.activation(out=tr, in_=rp[:], func=AF.Tanh, scale=0.5)
            nc.tensor.matmul(zp[:], wx[:, H:2 * H], xT[:, sl], start=True, stop=False)
            mm(zp[:], _EZ, hB, False, True)
            tz = temps.tile([H, BL], bf, tag="tz")
            nc.scalar.activation(out=tz, in_=zp[:], func=AF.Tanh, scale=-0.5)
            nc.tensor.matmul(ap_[:], wx[:, 2 * H:3 * H], xT[:, sl], start=True, stop=False)
            mm(ap_[:], _EA, hB, False, True)
            mm(bp[:], _EB, hB, True, True)
            pp = act_pp(tz)
            dve_chain(rp, zp, ap_, bp, tr, tz, pp)
            if t == l_steps - 1:
                nc.gpsimd.tensor_mul(out=oB[0:H, :], in0=m63, in1=h64)

        # ================= decoder =================
        for t in range(t_steps):
            rp = p_r.tile([H, BL], f32, tag="r")
            zp = p_z.tile([H, BL], f32, tag="z")
            ap_ = p_a.tile([H, BL], f32, tag="a")
            bp = p_b.tile([H, BL], f32, tag="b")
            if t == 0:
                mm(rp[:], _OR, oB, True, False)
                mm(rp[:], _ER, hB, False, True)
                tr = temps.tile([H, BL], bf, tag="tr")
                nc.scalar.activation(out=tr, in_=rp[:], func=AF.Tanh, scale=0.5)
                mm(zp[:], _OZ, oB, True, False)
                mm(zp[:], _EZ, hB, False, True)
                tz = temps.tile([H, BL], bf, tag="tz")
                nc.scalar.activation(out=tz, in_=zp[:], func=AF.Tanh, scale=-0.5)
                mm(ap_[:], _OA, oB, True, False)
                mm(ap_[:], _EA, hB, False, True)
                mm(bp[:], _EB, hB, True, True)
                pp = act_pp(tz)
                dve_chain(rp, zp, ap_, bp, tr, tz, pp)
                nc.vector.tensor_copy(out=oB[0:H, :], in_=h64)
                continue
            lt = p_l.tile([H, BL], f32, tag="lt")
            yt = p_y.tile([BL, OUT], f32, tag="yt")
            # r group + tanh_r
            mm(rp[:], _DR, hB, True, False)
            mm(rp[:], _AR, attnB, False, True)
            tr = temps.tile([H, BL], bf, tag="tr")
            nc.scalar.activation(out=tr, in_=rp[:], func=AF.Tanh, scale=0.5)
            # logits for o(t-1) + exp
            nc.tensor.matmul(lt[:], wh[:, _WOL:_WOL + H], hB[:],
                             start=True, stop=False)
            nc.tensor.matmul(lt[:], wh[:, _WOL:_WOL + H], attnB[:],
                             start=False, stop=True)
            nc.scalar.activation(out=eeo[:, 0:BL], in_=lt[:], func=AF.Exp)
            # z group + tanh_z
            mm(zp[:], _DZ, hB, True, False)
            mm(zp[:], _AZ, attnB, False, True)
            tz = temps.tile([H, BL], bf, tag="tz")
            nc.scalar.activation(out=tz, in_=zp[:], func=AF.Tanh, scale=-0.5)
            pp = act_pp(tz)
            # B then A groups
            mm(bp[:], _EB, hB, True, True)
            mm(ap_[:], _DA, hB, True, False)
            mm(ap_[:], _AA, attnB, False, True)
            # eo on Pool; s-accumulate early (rec path)
            nc.gpsimd.tensor_mul(out=eeo[:, BL:2 * BL], in0=eeo[:, 0:BL],
                                 in1=oB[0:H, :])
            if t == 1:
                nc.tensor.matmul(sacc[:], wh[0:H, _I64:_I64 + H],
                                 eeo[:], start=True, stop=True,
                                 skip_group_check=True)
            else:
                nc.tensor.matmul(sacc[:, 0:BL], wh[0:H, _I64:_I64 + H],
                                 eeo[:, 0:BL], start=False, stop=True,
                                 skip_group_check=True)
            # y head for o(t-1)
            nc.tensor.matmul(yt[:], hB[:], wh[:, _WOY:_WOY + OUT],
                             start=True, stop=False)
            nc.tensor.matmul(yt[:], attnB[:], wh[:, _WOY:_WOY + OUT],
                             start=False, stop=True)
            if t > 1:
                nc.tensor.matmul(sacc[:, BL:2 * BL], wh[0:H, _I64:_I64 + H],
                                 eeo[:, BL:2 * BL], start=False, stop=True,
                                 skip_group_check=True)
            # DVE chain then attention tail
            dve_chain(rp, zp, ap_, bp, tr, tz, pp)
            rec = temps.tile([H, BL], f32, tag="rec")
            nc.vector.reciprocal_approx_fast(out=rec, in_=sacc[:, 0:BL])
            nc.vector.tensor_mul(out=attnB[0:H, :], in0=sacc[:, BL:2 * BL],
                                 in1=rec)
            nc.gpsimd.tensor_add(out=oB[0:H, :], in0=h64, in1=attnB[0:H, :])
            nc.scalar.copy(out=out_sb[:, (t - 1) * OUT:t * OUT], in_=yt[:])
        # final y for t = T-1
        yt = p_y.tile([BL, OUT], f32, tag="yt")
        nc.tensor.matmul(yt[:], hB[:], wh[:, _WOY:_WOY + OUT],
                         start=True, stop=False)
        nc.tensor.matmul(yt[:], attnB[:], wh[:, _WOY:_WOY + OUT],
                         start=False, stop=True)
        nc.scalar.copy(out=out_sb[:, (t_steps - 1) * OUT:t_steps * OUT],
                       in_=yt[:])

        nc.sync.dma_start(out=d_out[:], in_=out_sb)
    if compile:
        nc.compile()
    return nc


def _make_in_maps(inputs):
    x = np.asarray(inputs["x"], np.float32)
    lengths = np.asarray(inputs["lengths"])
    w = _prep_weights(inputs["Wih"], inputs["Whh"], inputs["bih"],
                      inputs["bhh"], inputs["Wf"], inputs["bf"],
                      inputs["Wa"], inputs["ba"])
    in_maps = []
    for c in range(NCORES):
        sl = slice(c * BL, (c + 1) * BL)
        xT, m63 = _prep_core(x[sl], lengths[sl])
        in_maps.append(dict(xT=xT, m63=m63, **w))
    return in_maps


def kernel(**inputs):
    global LAST_EXEC_NS, TRACE_DIR
    from concourse.bass_utils import run_bass_kernel_spmd
    t_steps = int(inputs.get("output_length", T))
    assert t_steps == T, f"hardcoded for output_length={T}, got {t_steps}"
    nc = build_nc()
    in_maps = _make_in_maps(inputs)
    kw = {}
    if TRACE:
        import tempfile
        TRACE_DIR = tempfile.mkdtemp(prefix="bass_trace_")
        kw = dict(trace=True, tmpdir=TRACE_DIR)
    res = None
    for attempt in range(3):
        try:
            res = run_bass_kernel_spmd(nc, in_maps, list(range(NCORES)), **kw)
            break
        except Exception:
            if attempt == 2:
                raise
    LAST_EXEC_NS = res.exec_time_ns
    outs = [np.asarray(res.results[c]["out"]).reshape(BL, T, OUT)
            for c in range(NCORES)]
    return np.concatenate(outs, axis=0)


# revision 23
# speedup vs baseline: 1.1288x; 1.0842x over previous
"""Trainium2 Bass kernel for nn_AutoregressiveGRUWithAttention.

Data-parallel over batch: 8 cores x 128 batch. Feature-on-partition layout.

v2 architecture (custom-DVE gate chain):
  r-gate:  tanh via custom DVE deg-7 Horner poly (gamma-normalized d7=-1,
           no clamp; preacts empirically |v|<0.9, fit region [0,1.75]).
  z-gate:  EXACT tanh on ACT (exp_and_others table), scale=-0.5 in-op;
           encoder length-mask rides the z preact (+BIG) -> tanh(-30) = -1
           exactly freezes h past each sequence end.
           zc2 = 0.5*K_N*(1+tz) (ACT affine) feeds the fused n-gate op.
  n-gate:  yv = CLAMP_ADD(A, tr*B') then m = TANH7M(yv, zc2) where TANH7M
           is the factored deg-7 form u*((z-a)^2+b^2)*(1-z)*Src1 with the
           real root normalized to 1 (weights pre-scaled by G_N) and the
           leading coeff K_N folded into zc2. m = 0.5*(1+tz)*n directly.
  mix:     pp = 0.5*(1-tz)*h via custom ZC_MUL on DVE; h' = m + pp.
  Attention: streamed unnormalized softmax. logits/y matmuls run on h and
  attn parts separately (o = h + attn never blocks them); e=exp on ACT
  (bf16), eo = e*o on Pool; s|AC accumulate in a persistent PSUM bank via
  two PE identity-matmuls; rec = reciprocal_approx_fast(s); attnB = AC*rec.
  PSUM column-packed banks: rz = [r | z], ab = [A | B'] (all partitions
  0:64 so every elementwise consumer is lane-legal).
"""
import numpy as np
import ml_dtypes

B, L, T, IN, H, OUT = 1024, 64, 128, 13, 64, 13
NCORES, BL = 8, 128
BIG = 60.0
BF16 = ml_dtypes.bfloat16

# ---- fitted polynomial constants (see sim.py) ----
# r,z gates: EXACT tanh on ACT. n-gate deg-7 factored on [0,1.6]:
G_N = 0.4280260088298112
A_N = 0.1851027405103682
B2_N = 0.4952995825046665
K_N = 4.3954206142704955
CLAMP_N = G_N * 1.6

# wh pack column offsets [65 x _WH_COLS] (bf16); all lhsT [K=65, 64]
_ER, _EZ, _EA, _EB = 0, 64, 128, 192
_DR, _DZ, _DA = 256, 320, 384
_AR, _AZ, _AA = 448, 512, 576
_OR, _OZ, _OA = 640, 704, 768
_WOL, _WOY, _I64 = 832, 896, 909
_WH_COLS = 973
_WX_COLS = 192  # [XR 64 | XZ 64 | XA 64], K = 14 (13 features + invm row)

LAST_EXEC_NS = None
TRACE = False
TRACE_DIR = None
WARM_DUMMIES = 0

_OPS = {}


def _register_ops():
    """Define + register the custom DVE ops (idempotent per process)."""
    global _OPS
    if _OPS:
        return _OPS
    import concourse.dve_ops as dve_ops
    from concourse.dve_spec import (Spec, Src0, Src1, C0, C1, C2, Zero, One,
                                    maxx, minn, sq, lower, _has_src1)
    from concourse.dve_uop import DveOpSpec

    # out = (x*((z-C0)^2 + C1)) * (1-z), z = x^2: factored deg-7 tanh/K_N
    # (real root normalized to 1 via G_N weight prescale); K_N rides the
    # h'-mix stt scalar.
    _zm = sq(Src0)
    TANH7P = Spec(
        body=(Src0 * (sq(_zm - C0) + C1)) * (One - _zm),
        reference=lambda in0, in1, s0, s1, imm2: (
            lambda u, z: u * ((z - s0) ** 2 + s1) * (1.0 - z))(
                in0.astype(np.float32), in0.astype(np.float32) ** 2),
    )

    CLAMP_ADD = Spec(
        body=minn(maxx(Src0 + Src1, Zero - C2), C2),
        reference=lambda in0, in1, s0, s1, imm2: np.clip(
            in0.astype(np.float32) + in1, -imm2, imm2),
    )

    # out = (C0 - Src0) * Src1 * C1     (pp = 0.5*(1-tz)*h)
    ZC_MUL = Spec(
        body=(C0 - Src0) * Src1 * C1,
        reference=lambda in0, in1, s0, s1, imm2: (
            (s0 - in0.astype(np.float32)) * in1 * s1),
    )

    specs = {"ANT_TANH7P": TANH7P,
             "ANT_CLAMP_ADD": CLAMP_ADD, "ANT_ZC_MUL": ZC_MUL}
    ops = {}
    for name, spec in specs.items():
        if name in dve_ops._SUB_OPCODE_FOR_NAME:
            ops[name] = next(o for o in dve_ops.OPS if o.name == name)
            continue
        row = max(dve_ops._SUB_OPCODE_FOR_NAME.values()) + 1
        assert row < 0x20, "custom-DVE row overflow"
        dve_ops._SUB_OPCODE_FOR_NAME[name] = row
        shas = {}
        for ver in ("v3", "v4"):
            try:
                uops = lower(spec, ver=ver)
            except Exception:
                continue
            shas[ver] = DveOpSpec(name=name, opcode=row, uops=uops,
                                  rd1_en=_has_src1(spec)).sha(ver)
        op = dve_ops.DveOp(name, spec, False, shas,
                           perf_en={"v3": True, "v4": True}
                           if name == "ANT_TANH7P" else {})
        dve_ops.OPS.append(op)
        dve_ops.CUSTOM_DVE_SPECS[name] = spec
        ops[name] = op
    _OPS = ops
    return ops


def _prep_weights(Wih, Whh, bih, bhh, Wf, bf, Wa, ba):
    f8 = np.float64
    Wih, Whh, bih, bhh, Wf, bf, Wa, ba = [np.asarray(a, f8) for a in
                                          (Wih, Whh, bih, bhh, Wf, bf, Wa, ba)]
    Wr, Wz, Wn = Wih[0:H], Wih[H:2 * H], Wih[2 * H:3 * H]
    Ur, Uz, Un = Whh[0:H], Whh[H:2 * H], Whh[2 * H:3 * H]
    br_i, bz_i, bn_i = bih[0:H], bih[H:2 * H], bih[2 * H:3 * H]
    br_h, bz_h, bn_h = bhh[0:H], bhh[H:2 * H], bhh[2 * H:3 * H]
    CR, CZ, CA = Wf.T @ Wr.T, Wf.T @ Wz.T, Wf.T @ Wn.T
    cr_b, cz_b, ca_b = bf @ Wr.T, bf @ Wz.T, bf @ Wn.T

    def blk(w, bias, scale):
        m = np.zeros((H + 1, H), f8)
        m[0:H] = scale * w
        m[H] = scale * bias
        return m

    wh = np.zeros((H + 1, _WH_COLS), f8)
    wh[:, _ER:_ER + H] = blk(Ur.T, br_i + br_h, 1.0)
    wh[:, _EZ:_EZ + H] = blk(Uz.T, bz_i + bz_h, 1.0)
    wh[:, _EA:_EA + H] = blk(0.5 * Un.T, bn_i + 0.5 * bn_h, G_N)
    wh[:, _EB:_EB + H] = blk(0.5 * Un.T, 0.5 * bn_h, G_N)
    wh[:, _DR:_DR + H] = blk(Ur.T + CR, br_i + br_h + cr_b, 1.0)
    wh[:, _DZ:_DZ + H] = blk(Uz.T + CZ, bz_i + bz_h + cz_b, 1.0)
    wh[:, _DA:_DA + H] = blk(0.5 * Un.T + CA, bn_i + ca_b + 0.5 * bn_h, G_N)
    wh[:, _AR:_AR + H] = blk(CR, 0 * cr_b, 1.0)
    wh[:, _AZ:_AZ + H] = blk(CZ, 0 * cz_b, 1.0)
    wh[:, _AA:_AA + H] = blk(CA, 0 * ca_b, G_N)
    wh[:, _OR:_OR + H] = blk(CR, cr_b, 1.0)
    wh[:, _OZ:_OZ + H] = blk(CZ, cz_b, 1.0)
    wh[:, _OA:_OA + H] = blk(CA, ca_b, G_N)
    wh[:, _WOL:_WOL + H] = blk(Wa.T, ba, 1.0)
    wh[0:H, _WOY:_WOY + OUT] = Wf.T
    wh[H, _WOY:_WOY + OUT] = bf
    wh[0:H, _I64:_I64 + H] = np.eye(H)

    wx = np.zeros((IN + 1, _WX_COLS), f8)
    wx[0:IN, 0:H] = Wr.T
    wx[0:IN, H:2 * H] = Wz.T
    wx[IN, H:2 * H] = BIG                      # invm mask row (z only)
    wx[0:IN, 2 * H:3 * H] = G_N * Wn.T

    return dict(wh=np.ascontiguousarray(wh, BF16),
                wx=np.ascontiguousarray(wx, BF16))


def _prep_core(x_core, len_core):
    x_core = np.asarray(x_core, np.float32)
    xT = np.zeros((IN + 1, L, BL), np.float32)
    xT[0:IN] = np.transpose(x_core, (2, 1, 0))
    valid = (np.arange(L)[:, None] < np.asarray(len_core)[None, :])
    xT[IN] = 1.0 - valid.astype(np.float32)
    m63 = valid[L - 1].astype(np.float32)
    m63bc = np.ascontiguousarray(np.broadcast_to(m63, (H, BL)), BF16)
    return (np.ascontiguousarray(xT.reshape(IN + 1, L * BL), BF16), m63bc)


def build_nc(l_steps=L, t_steps=T, compile=True):
    ops = _register_ops()
    TANH7P = ops["ANT_TANH7P"]
    CLAMP_ADD, ZC_MUL = ops["ANT_CLAMP_ADD"], ops["ANT_ZC_MUL"]
    import concourse.bacc as bacc
    import concourse.tile as tile
    from concourse import mybir
    from contextlib import ExitStack

    f32 = mybir.dt.float32
    bf = mybir.dt.bfloat16
    AF = mybir.ActivationFunctionType
    OP = mybir.AluOpType

    nc = bacc.Bacc("TRN2", target_bir_lowering=False, debug=False,
                   num_devices=NCORES)
    d_xT = nc.declare_dram_parameter("xT", [IN + 1, l_steps * BL], bf, isOutput=False)
    d_m63 = nc.declare_dram_parameter("m63", [H, BL], bf, isOutput=False)
    d_wh = nc.declare_dram_parameter("wh", [H + 1, _WH_COLS], bf, isOutput=False)
    d_wx = nc.declare_dram_parameter("wx", [IN + 1, _WX_COLS], bf, isOutput=False)
    d_out = nc.declare_dram_parameter("out", [BL, t_steps * OUT], f32, isOutput=True)

    with tile.TileContext(nc) as tc, ExitStack() as ctx:
        const = ctx.enter_context(tc.tile_pool(name="const", bufs=1))
        temps = ctx.enter_context(tc.tile_pool(name="temps", bufs=3))
        p_r = ctx.enter_context(tc.tile_pool(name="p_r", bufs=1, space="PSUM"))
        p_z = ctx.enter_context(tc.tile_pool(name="p_z", bufs=1, space="PSUM"))
        p_a = ctx.enter_context(tc.tile_pool(name="p_a", bufs=1, space="PSUM"))
        p_b = ctx.enter_context(tc.tile_pool(name="p_b", bufs=1, space="PSUM"))
        p_l = ctx.enter_context(tc.tile_pool(name="p_l", bufs=2, space="PSUM"))
        p_y = ctx.enter_context(tc.tile_pool(name="p_y", bufs=1, space="PSUM"))
        p_sacc = ctx.enter_context(tc.tile_pool(name="p_sacc", bufs=1, space="PSUM"))

        xT = const.tile([IN + 1, l_steps * BL], bf)
        m63 = const.tile([H, BL], bf)
        wh = const.tile([H + 1, _WH_COLS], bf)
        wx = const.tile([IN + 1, _WX_COLS], bf)
        hB = const.tile([H + 1, BL], bf)
        oB = const.tile([H + 1, BL], bf)
        attnB = const.tile([H + 1, BL], bf)
        eeo = const.tile([H, 2 * BL], bf)          # [e | eo]
        out_sb = const.tile([BL, t_steps * OUT], f32)
        b05 = const.tile([H, 1], f32)
        sacc = p_sacc.tile([H, 2 * BL], f32)       # [s | AC] persistent

        for dst, srcd in ((xT, d_xT), (m63, d_m63), (wh, d_wh), (wx, d_wx)):
            nc.sync.dma_start(out=dst, in_=srcd[:])

        nc.vector.memset(hB[:], 0.0)
        nc.vector.memset(hB[H:H + 1, :], 1.0)
        nc.vector.memset(oB[H:H + 1, :], 1.0)
        nc.vector.memset(attnB[:], 0.0)
        nc.vector.memset(b05, 0.5)
        h64 = hB[0:H, :]

        def mm(out_ap, col, rhs, start, stop):
            nc.tensor.matmul(out_ap, wh[:, col:col + H], rhs[:],
                             start=start, stop=stop)

        def dve_chain(rp, zp, ap_, bp, tr, tz):
            """t2 -> yv -> pp -> n' -> m~ -> h' (DVE, partitions 0:64)."""
            t2 = temps.tile([H, BL], f32, tag="t2")
            nc.vector.tensor_mul(out=t2, in0=tr, in1=bp[:])
            yv = temps.tile([H, BL], bf, tag="yv")
            nc.vector._custom_dve(CLAMP_ADD, out=yv, in0=ap_[:], in1=t2,
                                  imm2=CLAMP_N)
            pp = temps.tile([H, BL], bf, tag="pp")
            nc.vector._custom_dve(ZC_MUL, out=pp, in0=tz, in1=h64,
                                  s0=1.0, s1=0.5)
            nn = temps.tile([H, BL], bf, tag="nn")
            nc.vector._custom_dve(TANH7P, out=nn, in0=yv,
                                  s0=A_N, s1=B2_N)
            mt = temps.tile([H, BL], bf, tag="mt")
            nc.vector.scalar_tensor_tensor(out=mt, in0=tz, scalar=1.0,
                                           in1=nn, op0=OP.add, op1=OP.mult)
            nc.vector.scalar_tensor_tensor(out=h64, in0=mt, scalar=0.5 * K_N,
                                           in1=pp, op0=OP.mult, op1=OP.add)

        # ================= encoder =================
        for t in range(l_steps):
            rp = p_r.tile([H, BL], f32, tag="r")
            zp = p_z.tile([H, BL], f32, tag="z")
            ap_ = p_a.tile([H, BL], f32, tag="a")
            bp = p_b.tile([H, BL], f32, tag="b")
            sl = slice(t * BL, (t + 1) * BL)
            nc.tensor.matmul(rp[:], wx[:, 0:H], xT[:, sl], start=True, stop=False)
            mm(rp[:], _ER, hB, False, True)
            tr = temps.tile([H, BL], bf, tag="tr")
            nc.scalar.activation(out=tr, in_=rp[:], func=AF.Tanh, scale=0.5)
            nc.tensor.matmul(zp[:], wx[:, H:2 * H], xT[:, sl], start=True, stop=False)
            mm(zp[:], _EZ, hB, False, True)
            tz = temps.tile([H, BL], bf, tag="tz")
            nc.scalar.activation(out=tz, in_=zp[:], func=AF.Tanh, scale=-0.5)
            nc.tensor.matmul(ap_[:], wx[:, 2 * H:3 * H], xT[:, sl], start=True, stop=False)
            mm(ap_[:], _EA, hB, False, True)
            mm(bp[:], _EB, hB, True, True)
            dve_chain(rp, zp, ap_, bp, tr, tz)
            if t == l_steps - 1:
                nc.gpsimd.tensor_mul(out=oB[0:H, :], in0=m63, in1=h64)

        # ================= decoder =================
        for t in range(t_steps):
            rp = p_r.tile([H, BL], f32, tag="r")
            zp = p_z.tile([H, BL], f32, tag="z")
            ap_ = p_a.tile([H, BL], f32, tag="a")
            bp = p_b.tile([H, BL], f32, tag="b")
            if t == 0:
                mm(rp[:], _OR, oB, True, False)
                mm(rp[:], _ER, hB, False, True)
                tr = temps.tile([H, BL], bf, tag="tr")
                nc.scalar.activation(out=tr, in_=rp[:], func=AF.Tanh, scale=0.5)
                mm(zp[:], _OZ, oB, True, False)
                mm(zp[:], _EZ, hB, False, True)
                tz = temps.tile([H, BL], bf, tag="tz")
                nc.scalar.activation(out=tz, in_=zp[:], func=AF.Tanh, scale=-0.5)
                mm(ap_[:], _OA, oB, True, False)
                mm(ap_[:], _EA, hB, False, True)
                mm(bp[:], _EB, hB, True, True)
                dve_chain(rp, zp, ap_, bp, tr, tz)
                nc.vector.tensor_copy(out=oB[0:H, :], in_=h64)
                continue
            lt = p_l.tile([H, BL], f32, tag="lt")
            yt = p_y.tile([BL, OUT], f32, tag="yt")
            # r group + tanh_r
            mm(rp[:], _DR, hB, True, False)
            mm(rp[:], _AR, attnB, False, True)
            tr = temps.tile([H, BL], bf, tag="tr")
            nc.scalar.activation(out=tr, in_=rp[:], func=AF.Tanh, scale=0.5)
            # logits for o(t-1) + exp
            nc.tensor.matmul(lt[:], wh[:, _WOL:_WOL + H], hB[:],
                             start=True, stop=False)
            nc.tensor.matmul(lt[:], wh[:, _WOL:_WOL + H], attnB[:],
                             start=False, stop=True)
            nc.scalar.activation(out=eeo[:, 0:BL], in_=lt[:], func=AF.Exp)
            # z group + tanh_z
            mm(zp[:], _DZ, hB, True, False)
            mm(zp[:], _AZ, attnB, False, True)
            tz = temps.tile([H, BL], bf, tag="tz")
            nc.scalar.activation(out=tz, in_=zp[:], func=AF.Tanh, scale=-0.5)
            # B then A groups
            mm(bp[:], _EB, hB, True, True)
            mm(ap_[:], _DA, hB, True, False)
            mm(ap_[:], _AA, attnB, False, True)
            # eo on Pool; s-accumulate early (rec path)
            nc.gpsimd.tensor_mul(out=eeo[:, BL:2 * BL], in0=eeo[:, 0:BL],
                                 in1=oB[0:H, :])
            if t == 1:
                nc.tensor.matmul(sacc[:], wh[0:H, _I64:_I64 + H],
                                 eeo[:], start=True, stop=True,
                                 skip_group_check=True)
            else:
                nc.tensor.matmul(sacc[:, 0:BL], wh[0:H, _I64:_I64 + H],
                                 eeo[:, 0:BL], start=False, stop=True,
                                 skip_group_check=True)
            # y head for o(t-1)
            nc.tensor.matmul(yt[:], hB[:], wh[:, _WOY:_WOY + OUT],
                             start=True, stop=False)
            nc.tensor.matmul(yt[:], attnB[:], wh[:, _WOY:_WOY + OUT],
                             start=False, stop=True)
            if t > 1:
                nc.tensor.matmul(sacc[:, BL:2 * BL], wh[0:H, _I64:_I64 + H],
                                 eeo[:, BL:2 * BL], start=False, stop=True,
                                 skip_group_check=True)
            # DVE chain then attention tail
            dve_chain(rp, zp, ap_, bp, tr, tz)
            rec = temps.tile([H, BL], f32, tag="rec")
            nc.vector.reciprocal_approx_fast(out=rec, in_=sacc[:, 0:BL])
            nc.vector.tensor_mul(out=attnB[0:H, :], in0=sacc[:, BL:2 * BL],
                                 in1=rec)
            nc.gpsimd.tensor_add(out=oB[0:H, :], in0=h64, in1=attnB[0:H, :])
            nc.scalar.copy(out=out_sb[:, (t - 1) * OUT:t * OUT], in_=yt[:])
        # final y for t = T-1
        yt = p_y.tile([BL, OUT], f32, tag="yt")
        nc.tensor.matmul(yt[:], hB[:], wh[:, _WOY:_WOY + OUT],
                         start=True, stop=False)
        nc.tensor.matmul(yt[:], attnB[:], wh[:, _WOY:_WOY + OUT],
                         start=False, stop=True)
        nc.scalar.copy(out=out_sb[:, (t_steps - 1) * OUT:t_steps * OUT],
                       in_=yt[:])

        nc.sync.dma_start(out=d_out[:], in_=out_sb)
    if compile:
        nc.compile()
    return nc


def _make_in_maps(inputs):
    x = np.asarray(inputs["x"], np.float32)
    lengths = np.asarray(inputs["lengths"])
    w = _prep_weights(inputs["Wih"], inputs["Whh"], inputs["bih"],
                      inputs["bhh"], inputs["Wf"], inputs["bf"],
                      inputs["Wa"], inputs["ba"])
    in_maps = []
    for c in range(NCORES):
        sl = slice(c * BL, (c + 1) * BL)
        xT, m63 = _prep_core(x[sl], lengths[sl])
        in_maps.append(dict(xT=xT, m63=m63, **w))
    return in_maps


def kernel(**inputs):
    global LAST_EXEC_NS, TRACE_DIR
    from concourse.bass_utils import run_bass_kernel_spmd
    t_steps = int(inputs.get("output_length", T))
    assert t_steps == T, f"hardcoded for output_length={T}, got {t_steps}"
    nc = build_nc()
    in_maps = _make_in_maps(inputs)
    kw = {}
    if TRACE:
        import tempfile
        TRACE_DIR = tempfile.mkdtemp(prefix="bass_trace_")
        kw = dict(trace=True, tmpdir=TRACE_DIR)
    res = None
    for attempt in range(3):
        try:
            res = run_bass_kernel_spmd(nc, in_maps, list(range(NCORES)), **kw)
            break
        except Exception:
            if attempt == 2:
                raise
    LAST_EXEC_NS = res.exec_time_ns
    outs = [np.asarray(res.results[c]["out"]).reshape(BL, T, OUT)
            for c in range(NCORES)]
    return np.concatenate(outs, axis=0)
